# revision 26
# baseline (speedup 1.0000x reference)
"""AttentionOverlapLoss Trainium2 kernel.

Reference computation (per image, attn_map (B,224,224) f32, bboxes (B,4) i64):
    a   = (x - mn) / (mx - mn + eps)           # min-max normalize
    a   = a / (sum(a) + eps)                   # sum-to-one
    out = mean_b( sum(a * (1 - boxmask)) )

Algebraic reduction: with S = sum(x), Sbox = sum(x in box), N = H*W,
Nbox = box area, d = mx - mn + eps:
    sumA    = (S    - N   *mn)/d
    sumAbox = (Sbox - Nbox*mn)/d
    loss_i  = (sumA - sumAbox) / (sumA + eps)
So the device only needs 4 reductions per image: min, max, sum, box-sum.

Device strategy (pure data parallel, batch sharded 8 ways):
  * Each core streams its 256 images as 448 "row tiles" of (128 rows x 224
    cols); the flat row r = 128*s + p lives at partition p of tile s.
  * ScalarE casts each f32 tile group to bf16.
  * VectorE computes per-(partition,tile) min and max over the 224 cols with
    a bf16 tensor_tensor tree (2x mode) + final tensor_reduce.
  * TensorE computes, per tile, col-sums contracted against 4 indicator
    columns [rowbox(imgA), rowbox(imgB), is(imgA), is(imgB)] (a 128-row tile
    spans at most 2 images).  The image tile is the *stationary* operand
    (M=112 output partitions = image columns, two column halves).
  * Host applies the column-box mask to the tiny per-tile col-sum stats and
    finishes the scalar formula in float64.
"""

import numpy as np
import ml_dtypes

B, H, W = 2048, 224, 224
NCORES = 8
BL = B // NCORES            # 256 images per core
RPC = BL * H                # 57344 flat rows per core
P = 128                     # rows per tile
NT = RPC // P               # 448 tiles per core
TPG = 14                    # tiles per group (= 8 images per group);
                            # 14 beats 28 in the timeline model (finer
                            # DMA/compute pipelining, shorter ramp+tail)
NG = NT // TPG              # 32 groups
HW2 = W // 2                # 112 column half
NPIX = H * W
EPS = 1e-8

PROFILE = False             # set True (e.g. from test.py) to trace + time
LAST_RESULT = None          # BassKernelResults of the last run

_compiled = None


def _build_module(loop_k=1, parts="all", dma_groups=1):
    """loop_k > 1 wraps the steady-state body in a hardware For_i loop —
    benchmarking only; the graded path uses loop_k=1 (no loop).
    parts: ablation knob for benchmarking — "dma", "dma_cast",
    "dma_cast_trees", "dma_cast_pe", or "all"."""
    import contextlib

    import concourse.bacc as bacc
    import concourse.bass as bass
    import concourse.mybir as mybir
    from concourse import tile

    f32 = mybir.dt.float32
    bf16 = mybir.dt.bfloat16

    nc = bacc.Bacc("TRN2", target_bir_lowering=False, debug=False,
                   num_devices=NCORES)
    x = nc.declare_dram_parameter("x", [BL, H, W], f32, isOutput=False)
    rl = nc.declare_dram_parameter("rl", [P, NT * 4], bf16, isOutput=False)
    mins_o = nc.declare_dram_parameter("mins", [P, NT], f32, isOutput=True)
    maxs_o = nc.declare_dram_parameter("maxs", [P, NT], f32, isOutput=True)
    ps_o = nc.declare_dram_parameter("ps", [HW2, NG * TPG * 2 * 4], f32,
                                     isOutput=True)

    with tile.TileContext(nc) as tc:
        with (
            tc.tile_pool(name="const", bufs=1) as constp,
            tc.tile_pool(name="xg", bufs=(3 if dma_groups == 1 else 2)) as xgp,
            tc.tile_pool(name="xb", bufs=2) as xbp,
            tc.tile_pool(name="tree", bufs=2) as treep,
            tc.tile_pool(name="psum", bufs=2, space=bass.MemorySpace.PSUM) as psump,
        ):
            do_pe = parts in ("all", "dma_cast_pe")
            do_trees = parts in ("all", "dma_cast_trees")
            rlt = minstat = maxstat = sbps = None
            if do_pe:
                rlt = constp.tile([P, NT * 4], bf16, tag="rlt")
                sbps = constp.tile([HW2, NG * TPG * 2 * 4], f32, tag="sbps")
                nc.sync.dma_start(rlt[:], rl[:])
            if do_trees:
                minstat = constp.tile([P, NT], f32, tag="minstat")
                maxstat = constp.tile([P, NT], f32, tag="maxstat")
            xview = (x[:].rearrange("b h w -> (b h) w")
                         .rearrange("(s p) w -> p s w", p=P))
            def group_body(g, xg_ap):
                if parts == "dma":
                    return
                xb = xbp.tile([P, TPG, W], bf16, tag="xb")
                nc.scalar.copy(xb[:], xg_ap)

                if do_pe:
                    pg = psump.tile([HW2, TPG, 2, 4], f32)
                    for t in range(TPG):
                        s = g * TPG + t
                        nc.tensor.matmul(pg[:, t, 0, :], xb[:, t, 0:HW2],
                                         rlt[:, 4 * s:4 * s + 4],
                                         start=True, stop=True)
                        nc.tensor.matmul(pg[:, t, 1, :], xb[:, t, HW2:W],
                                         rlt[:, 4 * s:4 * s + 4],
                                         start=True, stop=True)
                    gsz = TPG * 2 * 4
                    nc.scalar.copy(sbps[:, g * gsz:(g + 1) * gsz], pg[:])

                if parts in ("dma_cast", "dma_cast_pe"):
                    return
                # min/max trees over the 224 cols: 224->112->56->28->14,
                # then reduce the last 14.
                for opname, stat in (("min", minstat), ("max", maxstat)):
                    op = getattr(mybir.AluOpType, opname)
                    t1 = treep.tile([P, TPG, 112], bf16, tag=f"t1{opname}")
                    nc.vector.tensor_tensor(t1[:], xb[:, :, 0:112],
                                            xb[:, :, 112:224], op)
                    t2 = treep.tile([P, TPG, 56], bf16, tag=f"t2{opname}")
                    nc.vector.tensor_tensor(t2[:], t1[:, :, 0:56],
                                            t1[:, :, 56:112], op)
                    t3 = treep.tile([P, TPG, 28], bf16, tag=f"t3{opname}")
                    nc.vector.tensor_tensor(t3[:], t2[:, :, 0:28],
                                            t2[:, :, 28:56], op)
                    t4 = treep.tile([P, TPG, 14], bf16, tag=f"t4{opname}")
                    nc.vector.tensor_tensor(t4[:], t3[:, :, 0:14],
                                            t3[:, :, 14:28], op)
                    nc.vector.tensor_reduce(
                        stat[:, g * TPG:(g + 1) * TPG], t4[:],
                        axis=mybir.AxisListType.X, op=op)

            loop_cm = (tc.For_i(0, loop_k, 1) if loop_k > 1
                       else contextlib.nullcontext())
            with loop_cm:
                for gg in range(NG // dma_groups):
                    xg = xgp.tile([P, dma_groups * TPG, W], f32, tag="xg")
                    nc.sync.dma_start(
                        xg[:],
                        xview[:, gg * dma_groups * TPG:
                              (gg + 1) * dma_groups * TPG, :])
                    for h in range(dma_groups):
                        group_body(gg * dma_groups + h,
                                   xg[:, h * TPG:(h + 1) * TPG, :])

            if do_trees:
                nc.sync.dma_start(mins_o[:], minstat[:])
                nc.sync.dma_start(maxs_o[:], maxstat[:])
            if do_pe:
                nc.sync.dma_start(ps_o[:], sbps[:])

    nc.compile()
    return nc


def _build_module_v2(loop_k=1, parts="all"):
    """Layout v2: partition p of group g holds the 28 consecutive rows
    [g*3584 + 28p, +28) — every DMA partition-line is one contiguous 25 KiB
    HBM read, and each partition belongs to exactly one image (p // 8).
    The PE contracts each tile-column t against 32 indicator columns
    ([rbox(img 0..15), is(img 0..15)]) and accumulates over t in PSUM, so the
    per-group stats shrink to (112, 2, 32)."""
    import contextlib

    import concourse.bacc as bacc
    import concourse.bass as bass
    import concourse.mybir as mybir
    from concourse import tile

    f32 = mybir.dt.float32
    bf16 = mybir.dt.bfloat16
    IPG = BL // NG              # 16 images per group

    nc = bacc.Bacc("TRN2", target_bir_lowering=False, debug=False,
                   num_devices=NCORES)
    x = nc.declare_dram_parameter("x", [BL, H, W], f32, isOutput=False)
    rl = nc.declare_dram_parameter("rl", [P, NG * TPG * 2 * IPG], bf16,
                                   isOutput=False)
    mins_o = nc.declare_dram_parameter("mins", [P, NT], f32, isOutput=True)
    maxs_o = nc.declare_dram_parameter("maxs", [P, NT], f32, isOutput=True)
    ps_o = nc.declare_dram_parameter("ps", [HW2, NG * 2 * 2 * IPG], f32,
                                     isOutput=True)

    with tile.TileContext(nc) as tc:
        with (
            tc.tile_pool(name="const", bufs=1) as constp,
            tc.tile_pool(name="xg", bufs=3) as xgp,
            tc.tile_pool(name="xb", bufs=2) as xbp,
            tc.tile_pool(name="tree", bufs=2) as treep,
            tc.tile_pool(name="psum", bufs=2, space=bass.MemorySpace.PSUM) as psump,
        ):
            do_pe = parts in ("all", "dma_cast_pe")
            do_trees = parts in ("all", "dma_cast_trees")
            rlt = minstat = maxstat = sbps = None
            if do_pe:
                rlt = constp.tile([P, NG * TPG * 2 * IPG], bf16, tag="rlt")
                sbps = constp.tile([HW2, NG * 2 * 2 * IPG], f32, tag="sbps")
                nc.sync.dma_start(rlt[:], rl[:])
            if do_trees:
                minstat = constp.tile([P, NT], f32, tag="minstat")
                maxstat = constp.tile([P, NT], f32, tag="maxstat")
            xview = (x[:].rearrange("b h w -> (b h) w")
                         .rearrange("(g p r) w -> p g (r w)", g=NG, p=P))

            def group_body(g):
                xg = xgp.tile([P, TPG, W], f32, tag="xg")
                nc.sync.dma_start(
                    xg[:], xview[:, g, :].rearrange("p (r w) -> p r w", w=W))
                if parts == "dma":
                    return
                xb = xbp.tile([P, TPG, W], bf16, tag="xb")
                nc.scalar.copy(xb[:], xg[:])

                if do_pe:
                    NJ = 2 * IPG
                    pg = psump.tile([HW2, 2, NJ], f32, tag="pg")
                    for half, csl in ((0, slice(0, HW2)), (1, slice(HW2, W))):
                        for t in range(TPG):
                            base = (g * TPG + t) * NJ
                            nc.tensor.matmul(pg[:, half, :], xb[:, t, csl],
                                             rlt[:, base:base + NJ],
                                             start=(t == 0),
                                             stop=(t == TPG - 1))
                    gsz = 2 * NJ
                    nc.scalar.copy(sbps[:, g * gsz:(g + 1) * gsz], pg[:])

                if parts in ("dma_cast", "dma_cast_pe"):
                    return
                for opname, stat in (("min", minstat), ("max", maxstat)):
                    op = getattr(mybir.AluOpType, opname)
                    t1 = treep.tile([P, TPG, 112], bf16, tag=f"t1{opname}")
                    nc.vector.tensor_tensor(t1[:], xb[:, :, 0:112],
                                            xb[:, :, 112:224], op)
                    t2 = treep.tile([P, TPG, 56], bf16, tag=f"t2{opname}")
                    nc.vector.tensor_tensor(t2[:], t1[:, :, 0:56],
                                            t1[:, :, 56:112], op)
                    t3 = treep.tile([P, TPG, 28], bf16, tag=f"t3{opname}")
                    nc.vector.tensor_tensor(t3[:], t2[:, :, 0:28],
                                            t2[:, :, 28:56], op)
                    t4 = treep.tile([P, TPG, 14], bf16, tag=f"t4{opname}")
                    nc.vector.tensor_tensor(t4[:], t3[:, :, 0:14],
                                            t3[:, :, 14:28], op)
                    nc.vector.tensor_reduce(
                        stat[:, g * TPG:(g + 1) * TPG], t4[:],
                        axis=mybir.AxisListType.X, op=op)

            loop_cm = (tc.For_i(0, loop_k, 1) if loop_k > 1
                       else contextlib.nullcontext())
            with loop_cm:
                for g in range(NG):
                    group_body(g)

            if do_trees:
                nc.sync.dma_start(mins_o[:], minstat[:])
                nc.sync.dma_start(maxs_o[:], maxstat[:])
            if do_pe:
                nc.sync.dma_start(ps_o[:], sbps[:])

    nc.compile()
    return nc


def _build_module_v3(loop_k=1, parts="all", tpg=28, subsplit=1,
                     tail_half_trees=False):
    """Layout v3 (same DMA layout as v2: partition p of group g holds TPG
    consecutive rows starting at g*P*TPG + p*TPG, i.e. one contiguous
    TPG*224*4-byte HBM read per partition line).  With TPG=28 every
    partition line belongs to exactly ONE image (img = g*IPG + p//8), so:

      * ScalarE casts the group to bf16 with accum_out => per-(p,g) SUM.
      * VectorE computes per-(p,g) min/max with a single fused
        tensor_tensor_reduce per stat (halves elementwise, then reduce).
      * TensorE contracts each tile-column t against IPG bf16 box-row
        indicator columns (streamed per group), accumulating in PSUM =>
        per-(image, column) box-row sums.
    Host reduces the 8 partition lines per image and applies the column
    box mask."""
    import contextlib

    import concourse.bacc as bacc
    import concourse.bass as bass
    import concourse.mybir as mybir
    from concourse import tile

    f32 = mybir.dt.float32
    bf16 = mybir.dt.bfloat16
    TPGL = tpg
    NGL = NT // TPGL                # groups per core
    IPG = BL // NGL                 # images per group
    HALF = TPGL // 2

    nc = bacc.Bacc("TRN2", target_bir_lowering=False, debug=False,
                   num_devices=NCORES)
    x = nc.declare_dram_parameter("x", [BL, H, W], f32, isOutput=False)
    rb = nc.declare_dram_parameter("rb", [P, NGL * TPGL * IPG], bf16,
                                   isOutput=False)
    mins_o = nc.declare_dram_parameter("mins", [P, NGL], f32, isOutput=True)
    maxs_o = nc.declare_dram_parameter("maxs", [P, NGL], f32, isOutput=True)
    SPL = max(subsplit, 1)
    sums_o = nc.declare_dram_parameter("sums", [P, NGL * SPL], f32,
                                       isOutput=True)
    ps_o = nc.declare_dram_parameter("ps", [IPG, NGL * W], f32, isOutput=True)
    t1_o = None
    if parts == "gpsimd_only":
        t1_o = nc.declare_dram_parameter("t1o", [P, (tpg // 2) * W], bf16,
                                         isOutput=True)

    micro = parts in ("cast_only", "trees_only", "pe_only", "gpsimd_only")
    do_cast = parts != "dma"
    do_pe = parts in ("all", "dma_cast_pe")
    do_mm = parts in ("all", "dma_cast_trees")

    with tile.TileContext(nc) as tc:
        with (
            tc.tile_pool(name="const", bufs=1) as constp,
            tc.tile_pool(name="xg", bufs=3) as xgp,
            tc.tile_pool(name="xb", bufs=2) as xbp,
            tc.tile_pool(name="rbg", bufs=2) as rbp,
            tc.tile_pool(name="sc", bufs=2) as scp,
            tc.tile_pool(name="psum", bufs=2, space=bass.MemorySpace.PSUM) as psump,
        ):
            minstat = constp.tile([P, NGL], f32, tag="minstat")
            maxstat = constp.tile([P, NGL], f32, tag="maxstat")
            sumstat = constp.tile([P, NGL * SPL], f32, tag="sumstat")
            sbps = constp.tile([IPG, NGL * W], f32, tag="sbps")
            xview = (x[:].rearrange("b h w -> (b h) w")
                         .rearrange("(g p r) w -> p g (r w)", g=NGL, p=P))

            def tree_flat(src_flat, size, stat_ap, opname):
                # src_flat: [P, size] bf16; binary TT tree down to <=196,
                # then reduce into stat_ap [P, 1]
                op = getattr(mybir.AluOpType, opname)
                cur = src_flat
                sz = size
                while sz > 196:
                    half = sz // 2
                    nxt = scp.tile([P, half], bf16, tag=f"tf{half}{opname}")
                    nc.vector.tensor_tensor(
                        nxt[:], cur[:, 0:half], cur[:, half:2 * half], op)
                    cur = nxt[:]
                    sz = half
                nc.vector.tensor_reduce(
                    stat_ap, cur, axis=mybir.AxisListType.X, op=op)

            def group_body(g):
                split = subsplit if subsplit > 1 else 1
                step = TPGL // split
                xg = xgp.tile([P, TPGL, W], f32, tag="xg")
                xsrc = xview[:, g, :]
                xb = None
                if do_cast:
                    xb = xbp.tile([P, TPGL, W], bf16, tag="xb")
                if do_pe:
                    rbgt = rbp.tile([P, TPGL, IPG], bf16, tag="rbg")
                    base = g * TPGL * IPG
                    nc.sync.dma_start(rbgt[:],
                                      rb[:, base:base + TPGL * IPG]
                                      .rearrange("p (t j) -> p t j", j=IPG))
                for s_ in range(split):
                    rsl = slice(s_ * step, (s_ + 1) * step)
                    nc.sync.dma_start(
                        xg[:, rsl, :],
                        xsrc[:, s_ * step * W:(s_ + 1) * step * W]
                        .rearrange("p (r w) -> p r w", w=W))
                    if do_cast:
                        nc.scalar.activation(
                            xb[:, rsl, :], xg[:, rsl, :],
                            mybir.ActivationFunctionType.Copy,
                            accum_out=sumstat[:, g * split + s_:
                                              g * split + s_ + 1])
                if not do_cast:
                    return

                if do_mm:
                    # binary TT tree 6272 -> 3136 -> ... -> 196 -> reduce
                    # (tensor_tensor_reduce would be one op but NEFFs with
                    # it die with NRT_EXEC_UNIT_UNRECOVERABLE, so tree it
                    # is).  The last group instead runs one short tree per
                    # sub-cast block (DVE is FIFO, so interleave min/max
                    # per block): the tail only waits on the final
                    # sub-cast plus two ~1.2us trees instead of 7.5us.
                    if tail_half_trees and g == NGL - 1 and split > 1:
                        qmin = scp.tile([P, split], f32, tag="lhmin")
                        qmax = scp.tile([P, split], f32, tag="lhmax")
                        for s_ in range(split):
                            src = (xb[:, s_ * step:(s_ + 1) * step, :]
                                   .rearrange("p r w -> p (r w)"))
                            tree_flat(src, step * W, qmin[:, s_:s_ + 1],
                                      "min")
                            tree_flat(src, step * W, qmax[:, s_:s_ + 1],
                                      "max")
                        nc.vector.tensor_reduce(
                            minstat[:, g:g + 1], qmin[:],
                            axis=mybir.AxisListType.X,
                            op=mybir.AluOpType.min)
                        nc.vector.tensor_reduce(
                            maxstat[:, g:g + 1], qmax[:],
                            axis=mybir.AxisListType.X,
                            op=mybir.AluOpType.max)
                    else:
                        xbf = xb[:].rearrange("p r w -> p (r w)")
                        tree_flat(xbf, TPGL * W, minstat[:, g:g + 1], "min")
                        tree_flat(xbf, TPGL * W, maxstat[:, g:g + 1], "max")

                if do_pe:
                    pg = psump.tile([IPG, W], f32, tag="pg")
                    for t in range(TPGL):
                        nc.tensor.matmul(pg[:], rbgt[:, t, :], xb[:, t, :],
                                         start=(t == 0), stop=(t == TPGL - 1))
                    nc.scalar.copy(sbps[:, g * W:(g + 1) * W], pg[:])
                    nc.sync.dma_start(ps_o[:, g * W:(g + 1) * W],
                                      sbps[:, g * W:(g + 1) * W])

            def micro_body():
                # compute-only loops on resident tiles: measures pure
                # engine throughput for one stage, 16 "groups" per iter
                if parts == "cast_only":
                    for g in range(NGL):
                        xb = xbp.tile([P, TPGL, W], bf16, tag="xb")
                        nc.scalar.activation(
                            xb[:], mxg[:],
                            mybir.ActivationFunctionType.Copy,
                            accum_out=sumstat[:, g:g + 1])
                elif parts == "trees_only":
                    for g in range(NGL):
                        for opname, stat in (("min", minstat),
                                             ("max", maxstat)):
                            op = getattr(mybir.AluOpType, opname)
                            t1 = scp.tile([P, HALF * W], bf16,
                                          tag=f"t1{opname}")
                            nc.vector.tensor_tensor(
                                t1[:].rearrange("p (r w) -> p r w", w=W),
                                mxb[:, 0:HALF, :], mxb[:, HALF:TPGL, :], op)
                            cur = t1
                            sz = HALF * W
                            while sz > 196:
                                sz //= 2
                                nxt = scp.tile([P, sz], bf16,
                                               tag=f"t{sz}{opname}")
                                nc.vector.tensor_tensor(
                                    nxt[:], cur[:, 0:sz], cur[:, sz:2 * sz],
                                    op)
                                cur = nxt
                            nc.vector.tensor_reduce(
                                stat[:, g:g + 1], cur[:],
                                axis=mybir.AxisListType.X, op=op)
                elif parts == "gpsimd_only":
                    for g in range(NGL):
                        nc.gpsimd.tensor_tensor(
                            gt1[:].rearrange("p (r w) -> p r w", w=W),
                            mxb[:, 0:HALF, :], mxb[:, HALF:TPGL, :],
                            mybir.AluOpType.max)
                else:  # pe_only
                    for g in range(NGL):
                        pg = psump.tile([IPG, W], f32, tag="pg")
                        for t in range(TPGL):
                            nc.tensor.matmul(pg[:], mrb[:, t, :],
                                             mxb[:, t, :],
                                             start=(t == 0),
                                             stop=(t == TPGL - 1))
                        nc.vector.tensor_copy(sbps[:, g * W:(g + 1) * W],
                                              pg[:])

            gt1 = None
            if parts == "gpsimd_only":
                gt1 = constp.tile([P, HALF * W], bf16, tag="gt1")
            if micro:
                mxg = constp.tile([P, TPGL, W], f32, tag="mxg")
                nc.sync.dma_start(
                    mxg[:], xview[:, 0, :].rearrange("p (r w) -> p r w", w=W))
                mxb = constp.tile([P, TPGL, W], bf16, tag="mxb")
                nc.scalar.activation(mxb[:], mxg[:],
                                     mybir.ActivationFunctionType.Copy)
                mrb = constp.tile([P, TPGL, IPG], bf16, tag="mrb")
                nc.sync.dma_start(mrb[:],
                                  rb[:, 0:TPGL * IPG]
                                  .rearrange("p (t j) -> p t j", j=IPG))

            loop_cm = (tc.For_i(0, loop_k, 1) if loop_k > 1
                       else contextlib.nullcontext())
            with loop_cm:
                if micro:
                    micro_body()
                else:
                    for g in range(NGL):
                        group_body(g)

            if micro:
                # consume whatever the micro loop produced
                if parts == "cast_only":
                    nc.sync.dma_start(sums_o[:], sumstat[:])
                elif parts == "trees_only":
                    nc.sync.dma_start(mins_o[:], minstat[:])
                    nc.sync.dma_start(maxs_o[:], maxstat[:])
                elif parts == "gpsimd_only":
                    nc.sync.dma_start(t1_o[:], gt1[:])
                else:
                    nc.sync.dma_start(ps_o[:], sbps[:])
            if not micro and do_mm:
                nc.sync.dma_start(mins_o[:], minstat[:])
                nc.sync.dma_start(maxs_o[:], maxstat[:])
            if not micro and do_cast:
                nc.sync.dma_start(sums_o[:], sumstat[:])
            # ps is DMA'd out per group inside group_body

    nc.compile()
    return nc


def _host_prep_v3(x_np, bboxes, tpg=28):
    TPGL = tpg
    NGL = NT // TPGL
    IPG = BL // NGL
    bb = np.asarray(bboxes).astype(np.int64)
    x1 = np.clip(bb[:, 0], 0, W - 1)
    y1 = np.clip(bb[:, 1], 0, H - 1)
    x2 = np.clip(bb[:, 2], 0, W - 1)
    y2 = np.clip(bb[:, 3], 0, H - 1)
    yy = np.arange(H)
    xx = np.arange(W)
    rbox = (yy[None, :] >= y1[:, None]) & (yy[None, :] <= y2[:, None])  # (B,H)
    cbox = (xx[None, :] >= x1[:, None]) & (xx[None, :] <= x2[:, None])  # (B,W)

    # local flat row of (g,p,t) = g*P*TPG + p*TPG + t; every partition line
    # stays inside image  img_l = g*IPG + p//8,  rows 28*(p%8) + t.
    p_i = np.arange(P)[:, None, None]
    g_i = np.arange(NGL)[None, :, None]
    t_i = np.arange(TPGL)[None, None, :]
    img_l = g_i * IPG + p_i // (P // IPG)                 # (P, NG, TPG)
    within = (TPGL * (p_i % (P // IPG)) + t_i)
    onehot = (p_i[..., None] // (P // IPG) ==
              np.arange(IPG)[None, None, None, :])        # (P,1,1,IPG)

    in_maps = []
    for c in range(NCORES):
        Rm = rbox[img_l + c * BL, within]                 # (P, NG, TPG)
        rb_np = (Rm[..., None] & onehot).astype(ml_dtypes.bfloat16)
        in_maps.append({
            "x": np.ascontiguousarray(x_np[c * BL:(c + 1) * BL]),
            "rb": np.ascontiguousarray(
                rb_np.reshape(P, NGL * TPGL * IPG)),
        })
    return in_maps, rbox, cbox


def _host_combine_v3(results, rbox, cbox, tpg=28, subsplit=1):
    TPGL = tpg
    NGL = NT // TPGL
    IPG = BL // NGL
    PPI = P // IPG                                        # partitions/image
    SPL = max(subsplit, 1)
    mn = np.empty(B)
    mx = np.empty(B)
    S = np.empty(B)
    Sbox = np.empty(B)

    for c in range(NCORES):
        r = results[c]
        # stat[p, g] belongs to image g*IPG + p//PPI
        # -> [NG, IPG, PPI] with i = g*IPG + j
        mn[c * BL:(c + 1) * BL] = (
            r["mins"].T.reshape(NGL, IPG, PPI).min(2).reshape(BL))
        mx[c * BL:(c + 1) * BL] = (
            r["maxs"].T.reshape(NGL, IPG, PPI).max(2).reshape(BL))
        S[c * BL:(c + 1) * BL] = (
            r["sums"].astype(np.float64).reshape(P, NGL, SPL).sum(2).T
            .reshape(NGL, IPG, PPI).sum(2).reshape(BL))
        # ps[j, g*W + w] = box-row col sum for image g*IPG + j
        ps = (r["ps"].astype(np.float64).reshape(IPG, NGL, W)
              .transpose(1, 0, 2).reshape(BL, W))
        cb = cbox[c * BL:(c + 1) * BL]
        Sbox[c * BL:(c + 1) * BL] = (ps * cb).sum(1)

    nbox = rbox.sum(1).astype(np.float64) * cbox.sum(1).astype(np.float64)
    d = mx - mn + EPS
    sumA = (S - NPIX * mn) / d
    sumAbox = (Sbox - nbox * mn) / d
    loss = (sumA - sumAbox) / (sumA + EPS)
    return np.float32(loss.mean())


def _host_prep_v2(x_np, bboxes):
    IPG = BL // NG
    bb = np.asarray(bboxes).astype(np.int64)
    x1 = np.clip(bb[:, 0], 0, W - 1)
    y1 = np.clip(bb[:, 1], 0, H - 1)
    x2 = np.clip(bb[:, 2], 0, W - 1)
    y2 = np.clip(bb[:, 3], 0, H - 1)
    yy = np.arange(H)
    xx = np.arange(W)
    rbox = (yy[None, :] >= y1[:, None]) & (yy[None, :] <= y2[:, None])
    cbox = (xx[None, :] >= x1[:, None]) & (xx[None, :] <= x2[:, None])

    # local row of (g, p, t) is g*3584 + 28p + t; image-in-group = p // 8
    p_idx = np.arange(P)[:, None, None]
    g_idx = np.arange(NG)[None, :, None]
    t_idx = np.arange(TPG)[None, None, :]
    row = g_idx * (P * TPG) + p_idx * TPG + t_idx          # (P, NG, TPG)
    img_l = row // H                                       # local image
    within = row % H
    onehot = (p_idx[..., None] // 8 ==
              np.arange(IPG)[None, None, None, :])         # (P,1,1,IPG)

    in_maps = []
    for c in range(NCORES):
        Rm = rbox[img_l + c * BL, within]                  # (P, NG, TPG)
        full = np.zeros((P, NG, TPG, 2 * IPG), np.float32)
        full[..., :IPG] = onehot * Rm[..., None]
        full[..., IPG:] = np.broadcast_to(
            onehot, (P, NG, TPG, IPG)).astype(np.float32)
        rl_np = np.ascontiguousarray(
            full.reshape(P, NG * TPG * 2 * IPG)).astype(ml_dtypes.bfloat16)
        in_maps.append({
            "x": np.ascontiguousarray(x_np[c * BL:(c + 1) * BL]),
            "rl": rl_np,
        })
    return in_maps, rbox, cbox


def _host_combine_v2(results, rbox, cbox):
    IPG = BL // NG
    mn = np.empty(B)
    mx = np.empty(B)
    S = np.empty(B)
    Sbox = np.empty(B)

    for c in range(NCORES):
        r = results[c]
        # row g*3584 + 28p + t  ->  minstat[p, g*28 + t]
        rows_min = (r["mins"].reshape(P, NG, TPG).transpose(1, 0, 2)
                    .reshape(RPC))
        rows_max = (r["maxs"].reshape(P, NG, TPG).transpose(1, 0, 2)
                    .reshape(RPC))
        mn[c * BL:(c + 1) * BL] = rows_min.reshape(BL, H).min(1)
        mx[c * BL:(c + 1) * BL] = rows_max.reshape(BL, H).max(1)

        ps = r["ps"].reshape(HW2, NG, 2, 2 * IPG).astype(np.float64)
        pst = ps.transpose(1, 3, 2, 0)                     # (NG, 2*IPG, 2, HW2)
        box_part = pst[:, :IPG].reshape(BL, 2, HW2)        # l = g*IPG + j
        tot_part = pst[:, IPG:].reshape(BL, 2, HW2)
        cb = cbox[c * BL:(c + 1) * BL].reshape(BL, 2, HW2)
        Sbox[c * BL:(c + 1) * BL] = (box_part * cb).sum((1, 2))
        S[c * BL:(c + 1) * BL] = tot_part.sum((1, 2))

    nbox = rbox.sum(1).astype(np.float64) * cbox.sum(1).astype(np.float64)
    d = mx - mn + EPS
    sumA = (S - NPIX * mn) / d
    sumAbox = (Sbox - nbox * mn) / d
    loss = (sumA - sumAbox) / (sumA + EPS)
    return np.float32(loss.mean())


def _host_prep(x_np, bboxes):
    """Build per-core input maps."""
    bb = np.asarray(bboxes).astype(np.int64)
    x1 = np.clip(bb[:, 0], 0, W - 1)
    y1 = np.clip(bb[:, 1], 0, H - 1)
    x2 = np.clip(bb[:, 2], 0, W - 1)
    y2 = np.clip(bb[:, 3], 0, H - 1)
    yy = np.arange(H)
    xx = np.arange(W)
    rbox = (yy[None, :] >= y1[:, None]) & (yy[None, :] <= y2[:, None])  # (B,H)
    cbox = (xx[None, :] >= x1[:, None]) & (xx[None, :] <= x2[:, None])  # (B,W)

    img_of = np.repeat(np.arange(BL), H).reshape(NT, P)   # local img per row
    within = np.tile(np.arange(H), BL).reshape(NT, P)
    isA = img_of == img_of[:, 0:1]
    isB = ~isA

    in_maps = []
    for c in range(NCORES):
        gimg = img_of + c * BL
        Rm = rbox[gimg, within]                            # (NT, P) bool
        rl4 = np.stack([Rm & isA, Rm & isB, isA, isB], axis=2)  # (NT,P,4)
        rl_np = np.ascontiguousarray(
            rl4.transpose(1, 0, 2).reshape(P, NT * 4)).astype(ml_dtypes.bfloat16)
        in_maps.append({
            "x": np.ascontiguousarray(x_np[c * BL:(c + 1) * BL]),
            "rl": rl_np,
        })
    return in_maps, rbox, cbox


def _host_combine(results, rbox, cbox):
    mn = np.empty(B)
    mx = np.empty(B)
    S = np.zeros(B)
    Sbox = np.zeros(B)

    tiles = np.arange(NT)
    imgA_l = (P * tiles) // H                   # local first image of tile
    imgB_l = np.minimum(imgA_l + 1, BL - 1)     # clipped; contribution is 0
                                                # when the tile has no imgB rows
    cols = (np.arange(2)[:, None] * HW2 + np.arange(HW2)[None, :])  # (2,HW2)

    for c in range(NCORES):
        r = results[c]
        rows_min = r["mins"].T.reshape(RPC)     # row 128s+p -> mins[p,s]
        rows_max = r["maxs"].T.reshape(RPC)
        mn[c * BL:(c + 1) * BL] = rows_min.reshape(BL, H).min(1)
        mx[c * BL:(c + 1) * BL] = rows_max.reshape(BL, H).max(1)

        ps = r["ps"].reshape(HW2, NG, TPG, 2, 4).astype(np.float64)
        # -> (NT, 2, HW2, 4)
        pst = ps.transpose(1, 2, 3, 0, 4).reshape(NT, 2, HW2, 4)
        gA = imgA_l + c * BL
        gB = imgB_l + c * BL
        cA = cbox[gA][:, cols]                  # (NT, 2, HW2)
        cB = cbox[gB][:, cols]
        sboxA = (pst[..., 0] * cA).sum((1, 2))
        sboxB = (pst[..., 1] * cB).sum((1, 2))
        sA = pst[..., 2].sum((1, 2))
        sB = pst[..., 3].sum((1, 2))
        np.add.at(Sbox, gA, sboxA)
        np.add.at(Sbox, gB, sboxB)
        np.add.at(S, gA, sA)
        np.add.at(S, gB, sB)

    nbox = rbox.sum(1).astype(np.float64) * cbox.sum(1).astype(np.float64)
    d = mx - mn + EPS
    sumA = (S - NPIX * mn) / d
    sumAbox = (Sbox - nbox * mn) / d
    loss = (sumA - sumAbox) / (sumA + EPS)
    return np.float32(loss.mean())


BUILDER = "v3"
V3_CFG = dict(tpg=28, subsplit=4, tail_half_trees=True)


def kernel(attn_map, bboxes):
    global _compiled, LAST_RESULT
    from concourse.bass_utils import run_bass_kernel_spmd

    if _compiled is None:
        _compiled = (_build_module_v3(**V3_CFG) if BUILDER == "v3"
                     else _build_module())
    nc = _compiled

    x_np = np.ascontiguousarray(np.asarray(attn_map, dtype=np.float32))
    if BUILDER == "v3":
        in_maps, rbox, cbox = _host_prep_v3(x_np, bboxes, tpg=V3_CFG["tpg"])
    else:
        in_maps, rbox, cbox = _host_prep(x_np, bboxes)
    res = run_bass_kernel_spmd(nc, in_maps, list(range(NCORES)))
    LAST_RESULT = res
    if BUILDER == "v3":
        return _host_combine_v3(res.results, rbox, cbox,
                                tpg=V3_CFG["tpg"],
                                subsplit=V3_CFG["subsplit"])
    return _host_combine(res.results, rbox, cbox)



# revision 28
# speedup vs baseline: 1.2294x; 1.2294x over previous
"""AttentionOverlapLoss Trainium2 kernel.

Reference computation (per image, attn_map (B,224,224) f32, bboxes (B,4) i64):
    a   = (x - mn) / (mx - mn + eps)           # min-max normalize
    a   = a / (sum(a) + eps)                   # sum-to-one
    out = mean_b( sum(a * (1 - boxmask)) )

Algebraic reduction: with S = sum(x), Sbox = sum(x in box), N = H*W,
Nbox = box area, d = mx - mn + eps:
    sumA    = (S    - N   *mn)/d
    sumAbox = (Sbox - Nbox*mn)/d
    loss_i  = (sumA - sumAbox) / (sumA + eps)
So the device only needs 4 reductions per image: min, max, sum, box-sum.

Device strategy (pure data parallel, batch sharded 8 ways):
  * Each core streams its 256 images as 448 "row tiles" of (128 rows x 224
    cols); the flat row r = 128*s + p lives at partition p of tile s.
  * ScalarE casts each f32 tile group to bf16.
  * VectorE computes per-(partition,tile) min and max over the 224 cols with
    a bf16 tensor_tensor tree (2x mode) + final tensor_reduce.
  * TensorE computes, per tile, col-sums contracted against 4 indicator
    columns [rowbox(imgA), rowbox(imgB), is(imgA), is(imgB)] (a 128-row tile
    spans at most 2 images).  The image tile is the *stationary* operand
    (M=112 output partitions = image columns, two column halves).
  * Host applies the column-box mask to the tiny per-tile col-sum stats and
    finishes the scalar formula in float64.
"""

import numpy as np
import ml_dtypes

B, H, W = 2048, 224, 224
NCORES = 8
BL = B // NCORES            # 256 images per core
RPC = BL * H                # 57344 flat rows per core
P = 128                     # rows per tile
NT = RPC // P               # 448 tiles per core
TPG = 14                    # tiles per group (= 8 images per group);
                            # 14 beats 28 in the timeline model (finer
                            # DMA/compute pipelining, shorter ramp+tail)
NG = NT // TPG              # 32 groups
HW2 = W // 2                # 112 column half
NPIX = H * W
EPS = 1e-8

PROFILE = False             # set True (e.g. from test.py) to trace + time
LAST_RESULT = None          # BassKernelResults of the last run

_compiled = None


def _build_module(loop_k=1, parts="all", dma_groups=1):
    """loop_k > 1 wraps the steady-state body in a hardware For_i loop —
    benchmarking only; the graded path uses loop_k=1 (no loop).
    parts: ablation knob for benchmarking — "dma", "dma_cast",
    "dma_cast_trees", "dma_cast_pe", or "all"."""
    import contextlib

    import concourse.bacc as bacc
    import concourse.bass as bass
    import concourse.mybir as mybir
    from concourse import tile

    f32 = mybir.dt.float32
    bf16 = mybir.dt.bfloat16

    nc = bacc.Bacc("TRN2", target_bir_lowering=False, debug=False,
                   num_devices=NCORES)
    x = nc.declare_dram_parameter("x", [BL, H, W], f32, isOutput=False)
    rl = nc.declare_dram_parameter("rl", [P, NT * 4], bf16, isOutput=False)
    mins_o = nc.declare_dram_parameter("mins", [P, NT], f32, isOutput=True)
    maxs_o = nc.declare_dram_parameter("maxs", [P, NT], f32, isOutput=True)
    ps_o = nc.declare_dram_parameter("ps", [HW2, NG * TPG * 2 * 4], f32,
                                     isOutput=True)

    with tile.TileContext(nc) as tc:
        with (
            tc.tile_pool(name="const", bufs=1) as constp,
            tc.tile_pool(name="xg", bufs=(3 if dma_groups == 1 else 2)) as xgp,
            tc.tile_pool(name="xb", bufs=2) as xbp,
            tc.tile_pool(name="tree", bufs=2) as treep,
            tc.tile_pool(name="psum", bufs=2, space=bass.MemorySpace.PSUM) as psump,
        ):
            do_pe = parts in ("all", "dma_cast_pe")
            do_trees = parts in ("all", "dma_cast_trees")
            rlt = minstat = maxstat = sbps = None
            if do_pe:
                rlt = constp.tile([P, NT * 4], bf16, tag="rlt")
                sbps = constp.tile([HW2, NG * TPG * 2 * 4], f32, tag="sbps")
                nc.sync.dma_start(rlt[:], rl[:])
            if do_trees:
                minstat = constp.tile([P, NT], f32, tag="minstat")
                maxstat = constp.tile([P, NT], f32, tag="maxstat")
            xview = (x[:].rearrange("b h w -> (b h) w")
                         .rearrange("(s p) w -> p s w", p=P))
            def group_body(g, xg_ap):
                if parts == "dma":
                    return
                xb = xbp.tile([P, TPG, W], bf16, tag="xb")
                nc.scalar.copy(xb[:], xg_ap)

                if do_pe:
                    pg = psump.tile([HW2, TPG, 2, 4], f32)
                    for t in range(TPG):
                        s = g * TPG + t
                        nc.tensor.matmul(pg[:, t, 0, :], xb[:, t, 0:HW2],
                                         rlt[:, 4 * s:4 * s + 4],
                                         start=True, stop=True)
                        nc.tensor.matmul(pg[:, t, 1, :], xb[:, t, HW2:W],
                                         rlt[:, 4 * s:4 * s + 4],
                                         start=True, stop=True)
                    gsz = TPG * 2 * 4
                    nc.scalar.copy(sbps[:, g * gsz:(g + 1) * gsz], pg[:])

                if parts in ("dma_cast", "dma_cast_pe"):
                    return
                # min/max trees over the 224 cols: 224->112->56->28->14,
                # then reduce the last 14.
                for opname, stat in (("min", minstat), ("max", maxstat)):
                    op = getattr(mybir.AluOpType, opname)
                    t1 = treep.tile([P, TPG, 112], bf16, tag=f"t1{opname}")
                    nc.vector.tensor_tensor(t1[:], xb[:, :, 0:112],
                                            xb[:, :, 112:224], op)
                    t2 = treep.tile([P, TPG, 56], bf16, tag=f"t2{opname}")
                    nc.vector.tensor_tensor(t2[:], t1[:, :, 0:56],
                                            t1[:, :, 56:112], op)
                    t3 = treep.tile([P, TPG, 28], bf16, tag=f"t3{opname}")
                    nc.vector.tensor_tensor(t3[:], t2[:, :, 0:28],
                                            t2[:, :, 28:56], op)
                    t4 = treep.tile([P, TPG, 14], bf16, tag=f"t4{opname}")
                    nc.vector.tensor_tensor(t4[:], t3[:, :, 0:14],
                                            t3[:, :, 14:28], op)
                    nc.vector.tensor_reduce(
                        stat[:, g * TPG:(g + 1) * TPG], t4[:],
                        axis=mybir.AxisListType.X, op=op)

            loop_cm = (tc.For_i(0, loop_k, 1) if loop_k > 1
                       else contextlib.nullcontext())
            with loop_cm:
                for gg in range(NG // dma_groups):
                    xg = xgp.tile([P, dma_groups * TPG, W], f32, tag="xg")
                    nc.sync.dma_start(
                        xg[:],
                        xview[:, gg * dma_groups * TPG:
                              (gg + 1) * dma_groups * TPG, :])
                    for h in range(dma_groups):
                        group_body(gg * dma_groups + h,
                                   xg[:, h * TPG:(h + 1) * TPG, :])

            if do_trees:
                nc.sync.dma_start(mins_o[:], minstat[:])
                nc.sync.dma_start(maxs_o[:], maxstat[:])
            if do_pe:
                nc.sync.dma_start(ps_o[:], sbps[:])

    nc.compile()
    return nc


def _build_module_v2(loop_k=1, parts="all"):
    """Layout v2: partition p of group g holds the 28 consecutive rows
    [g*3584 + 28p, +28) — every DMA partition-line is one contiguous 25 KiB
    HBM read, and each partition belongs to exactly one image (p // 8).
    The PE contracts each tile-column t against 32 indicator columns
    ([rbox(img 0..15), is(img 0..15)]) and accumulates over t in PSUM, so the
    per-group stats shrink to (112, 2, 32)."""
    import contextlib

    import concourse.bacc as bacc
    import concourse.bass as bass
    import concourse.mybir as mybir
    from concourse import tile

    f32 = mybir.dt.float32
    bf16 = mybir.dt.bfloat16
    IPG = BL // NG              # 16 images per group

    nc = bacc.Bacc("TRN2", target_bir_lowering=False, debug=False,
                   num_devices=NCORES)
    x = nc.declare_dram_parameter("x", [BL, H, W], f32, isOutput=False)
    rl = nc.declare_dram_parameter("rl", [P, NG * TPG * 2 * IPG], bf16,
                                   isOutput=False)
    mins_o = nc.declare_dram_parameter("mins", [P, NT], f32, isOutput=True)
    maxs_o = nc.declare_dram_parameter("maxs", [P, NT], f32, isOutput=True)
    ps_o = nc.declare_dram_parameter("ps", [HW2, NG * 2 * 2 * IPG], f32,
                                     isOutput=True)

    with tile.TileContext(nc) as tc:
        with (
            tc.tile_pool(name="const", bufs=1) as constp,
            tc.tile_pool(name="xg", bufs=3) as xgp,
            tc.tile_pool(name="xb", bufs=2) as xbp,
            tc.tile_pool(name="tree", bufs=2) as treep,
            tc.tile_pool(name="psum", bufs=2, space=bass.MemorySpace.PSUM) as psump,
        ):
            do_pe = parts in ("all", "dma_cast_pe")
            do_trees = parts in ("all", "dma_cast_trees")
            rlt = minstat = maxstat = sbps = None
            if do_pe:
                rlt = constp.tile([P, NG * TPG * 2 * IPG], bf16, tag="rlt")
                sbps = constp.tile([HW2, NG * 2 * 2 * IPG], f32, tag="sbps")
                nc.sync.dma_start(rlt[:], rl[:])
            if do_trees:
                minstat = constp.tile([P, NT], f32, tag="minstat")
                maxstat = constp.tile([P, NT], f32, tag="maxstat")
            xview = (x[:].rearrange("b h w -> (b h) w")
                         .rearrange("(g p r) w -> p g (r w)", g=NG, p=P))

            def group_body(g):
                xg = xgp.tile([P, TPG, W], f32, tag="xg")
                nc.sync.dma_start(
                    xg[:], xview[:, g, :].rearrange("p (r w) -> p r w", w=W))
                if parts == "dma":
                    return
                xb = xbp.tile([P, TPG, W], bf16, tag="xb")
                nc.scalar.copy(xb[:], xg[:])

                if do_pe:
                    NJ = 2 * IPG
                    pg = psump.tile([HW2, 2, NJ], f32, tag="pg")
                    for half, csl in ((0, slice(0, HW2)), (1, slice(HW2, W))):
                        for t in range(TPG):
                            base = (g * TPG + t) * NJ
                            nc.tensor.matmul(pg[:, half, :], xb[:, t, csl],
                                             rlt[:, base:base + NJ],
                                             start=(t == 0),
                                             stop=(t == TPG - 1))
                    gsz = 2 * NJ
                    nc.scalar.copy(sbps[:, g * gsz:(g + 1) * gsz], pg[:])

                if parts in ("dma_cast", "dma_cast_pe"):
                    return
                for opname, stat in (("min", minstat), ("max", maxstat)):
                    op = getattr(mybir.AluOpType, opname)
                    t1 = treep.tile([P, TPG, 112], bf16, tag=f"t1{opname}")
                    nc.vector.tensor_tensor(t1[:], xb[:, :, 0:112],
                                            xb[:, :, 112:224], op)
                    t2 = treep.tile([P, TPG, 56], bf16, tag=f"t2{opname}")
                    nc.vector.tensor_tensor(t2[:], t1[:, :, 0:56],
                                            t1[:, :, 56:112], op)
                    t3 = treep.tile([P, TPG, 28], bf16, tag=f"t3{opname}")
                    nc.vector.tensor_tensor(t3[:], t2[:, :, 0:28],
                                            t2[:, :, 28:56], op)
                    t4 = treep.tile([P, TPG, 14], bf16, tag=f"t4{opname}")
                    nc.vector.tensor_tensor(t4[:], t3[:, :, 0:14],
                                            t3[:, :, 14:28], op)
                    nc.vector.tensor_reduce(
                        stat[:, g * TPG:(g + 1) * TPG], t4[:],
                        axis=mybir.AxisListType.X, op=op)

            loop_cm = (tc.For_i(0, loop_k, 1) if loop_k > 1
                       else contextlib.nullcontext())
            with loop_cm:
                for g in range(NG):
                    group_body(g)

            if do_trees:
                nc.sync.dma_start(mins_o[:], minstat[:])
                nc.sync.dma_start(maxs_o[:], maxstat[:])
            if do_pe:
                nc.sync.dma_start(ps_o[:], sbps[:])

    nc.compile()
    return nc


def _build_module_v3(loop_k=1, parts="all", tpg=28, subsplit=1,
                     tail_half_trees=False):
    """Layout v3 (same DMA layout as v2: partition p of group g holds TPG
    consecutive rows starting at g*P*TPG + p*TPG, i.e. one contiguous
    TPG*224*4-byte HBM read per partition line).  With TPG=28 every
    partition line belongs to exactly ONE image (img = g*IPG + p//8), so:

      * ScalarE casts the group to bf16 with accum_out => per-(p,g) SUM.
      * VectorE computes per-(p,g) min/max with a single fused
        tensor_tensor_reduce per stat (halves elementwise, then reduce).
      * TensorE contracts each tile-column t against IPG bf16 box-row
        indicator columns (streamed per group), accumulating in PSUM =>
        per-(image, column) box-row sums.
    Host reduces the 8 partition lines per image and applies the column
    box mask."""
    import contextlib

    import concourse.bacc as bacc
    import concourse.bass as bass
    import concourse.mybir as mybir
    from concourse import tile

    f32 = mybir.dt.float32
    bf16 = mybir.dt.bfloat16
    TPGL = tpg
    NGL = NT // TPGL                # groups per core
    IPG = BL // NGL                 # images per group
    HALF = TPGL // 2

    nc = bacc.Bacc("TRN2", target_bir_lowering=False, debug=False,
                   num_devices=NCORES)
    x = nc.declare_dram_parameter("x", [BL, H, W], f32, isOutput=False)
    rb = nc.declare_dram_parameter("rb", [P, NGL * TPGL * IPG], bf16,
                                   isOutput=False)
    mins_o = nc.declare_dram_parameter("mins", [P, NGL], f32, isOutput=True)
    maxs_o = nc.declare_dram_parameter("maxs", [P, NGL], f32, isOutput=True)
    SPL = max(subsplit, 1)
    sums_o = nc.declare_dram_parameter("sums", [P, NGL * SPL], f32,
                                       isOutput=True)
    ps_o = nc.declare_dram_parameter("ps", [IPG, NGL * W], f32, isOutput=True)
    t1_o = None
    if parts == "gpsimd_only":
        t1_o = nc.declare_dram_parameter("t1o", [P, (tpg // 2) * W], bf16,
                                         isOutput=True)

    micro = parts in ("cast_only", "trees_only", "pe_only", "gpsimd_only")
    do_cast = parts != "dma"
    do_pe = parts in ("all", "dma_cast_pe")
    do_mm = parts in ("all", "dma_cast_trees")

    with tile.TileContext(nc) as tc:
        with (
            tc.tile_pool(name="const", bufs=1) as constp,
            tc.tile_pool(name="xg", bufs=3) as xgp,
            tc.tile_pool(name="xb", bufs=2) as xbp,
            tc.tile_pool(name="rbg", bufs=2) as rbp,
            tc.tile_pool(name="sc", bufs=2) as scp,
            tc.tile_pool(name="psum", bufs=2, space=bass.MemorySpace.PSUM) as psump,
        ):
            minstat = constp.tile([P, NGL], f32, tag="minstat")
            maxstat = constp.tile([P, NGL], f32, tag="maxstat")
            sumstat = constp.tile([P, NGL * SPL], f32, tag="sumstat")
            sbps = constp.tile([IPG, NGL * W], f32, tag="sbps")
            xview = (x[:].rearrange("b h w -> (b h) w")
                         .rearrange("(g p r) w -> p g (r w)", g=NGL, p=P))

            def tree_flat(src_flat, size, stat_ap, opname):
                # src_flat: [P, size] bf16; binary TT tree down to <=196,
                # then reduce into stat_ap [P, 1]
                op = getattr(mybir.AluOpType, opname)
                cur = src_flat
                sz = size
                while sz > 196:
                    half = sz // 2
                    nxt = scp.tile([P, half], bf16, tag=f"tf{half}{opname}")
                    nc.vector.tensor_tensor(
                        nxt[:], cur[:, 0:half], cur[:, half:2 * half], op)
                    cur = nxt[:]
                    sz = half
                nc.vector.tensor_reduce(
                    stat_ap, cur, axis=mybir.AxisListType.X, op=op)

            def group_body(g):
                split = subsplit if subsplit > 1 else 1
                step = TPGL // split
                xg = xgp.tile([P, TPGL, W], f32, tag="xg")
                xsrc = xview[:, g, :]
                xb = None
                if do_cast:
                    xb = xbp.tile([P, TPGL, W], bf16, tag="xb")
                if do_pe:
                    rbgt = rbp.tile([P, TPGL, IPG], bf16, tag="rbg")
                    base = g * TPGL * IPG
                    nc.sync.dma_start(rbgt[:],
                                      rb[:, base:base + TPGL * IPG]
                                      .rearrange("p (t j) -> p t j", j=IPG))
                for s_ in range(split):
                    rsl = slice(s_ * step, (s_ + 1) * step)
                    nc.sync.dma_start(
                        xg[:, rsl, :],
                        xsrc[:, s_ * step * W:(s_ + 1) * step * W]
                        .rearrange("p (r w) -> p r w", w=W))
                    if do_cast:
                        nc.scalar.activation(
                            xb[:, rsl, :], xg[:, rsl, :],
                            mybir.ActivationFunctionType.Copy,
                            accum_out=sumstat[:, g * split + s_:
                                              g * split + s_ + 1])
                if not do_cast:
                    return

                if do_mm:
                    # binary TT tree 6272 -> 3136 -> ... -> 196 -> reduce
                    # (tensor_tensor_reduce would be one op but NEFFs with
                    # it die with NRT_EXEC_UNIT_UNRECOVERABLE, so tree it
                    # is).  The last group instead runs one short tree per
                    # sub-cast block (DVE is FIFO, so interleave min/max
                    # per block): the tail only waits on the final
                    # sub-cast plus two ~1.2us trees instead of 7.5us.
                    if tail_half_trees and g == NGL - 1 and split > 1:
                        qmin = scp.tile([P, split], f32, tag="lhmin")
                        qmax = scp.tile([P, split], f32, tag="lhmax")
                        for s_ in range(split):
                            src = (xb[:, s_ * step:(s_ + 1) * step, :]
                                   .rearrange("p r w -> p (r w)"))
                            tree_flat(src, step * W, qmin[:, s_:s_ + 1],
                                      "min")
                            tree_flat(src, step * W, qmax[:, s_:s_ + 1],
                                      "max")
                        nc.vector.tensor_reduce(
                            minstat[:, g:g + 1], qmin[:],
                            axis=mybir.AxisListType.X,
                            op=mybir.AluOpType.min)
                        nc.vector.tensor_reduce(
                            maxstat[:, g:g + 1], qmax[:],
                            axis=mybir.AxisListType.X,
                            op=mybir.AluOpType.max)
                    else:
                        xbf = xb[:].rearrange("p r w -> p (r w)")
                        tree_flat(xbf, TPGL * W, minstat[:, g:g + 1], "min")
                        tree_flat(xbf, TPGL * W, maxstat[:, g:g + 1], "max")

                if do_pe:
                    pg = psump.tile([IPG, W], f32, tag="pg")
                    for t in range(TPGL):
                        nc.tensor.matmul(pg[:], rbgt[:, t, :], xb[:, t, :],
                                         start=(t == 0), stop=(t == TPGL - 1))
                    # NOTE: do NOT dma ps out per group from nc.sync — the
                    # SP sequencer blocks on the DMA's wait (ACT copy) and
                    # stalls the next group's input DMAs behind it.
                    nc.scalar.copy(sbps[:, g * W:(g + 1) * W], pg[:])

            def micro_body():
                # compute-only loops on resident tiles: measures pure
                # engine throughput for one stage, 16 "groups" per iter
                if parts == "cast_only":
                    for g in range(NGL):
                        xb = xbp.tile([P, TPGL, W], bf16, tag="xb")
                        nc.scalar.activation(
                            xb[:], mxg[:],
                            mybir.ActivationFunctionType.Copy,
                            accum_out=sumstat[:, g:g + 1])
                elif parts == "trees_only":
                    for g in range(NGL):
                        for opname, stat in (("min", minstat),
                                             ("max", maxstat)):
                            op = getattr(mybir.AluOpType, opname)
                            t1 = scp.tile([P, HALF * W], bf16,
                                          tag=f"t1{opname}")
                            nc.vector.tensor_tensor(
                                t1[:].rearrange("p (r w) -> p r w", w=W),
                                mxb[:, 0:HALF, :], mxb[:, HALF:TPGL, :], op)
                            cur = t1
                            sz = HALF * W
                            while sz > 196:
                                sz //= 2
                                nxt = scp.tile([P, sz], bf16,
                                               tag=f"t{sz}{opname}")
                                nc.vector.tensor_tensor(
                                    nxt[:], cur[:, 0:sz], cur[:, sz:2 * sz],
                                    op)
                                cur = nxt
                            nc.vector.tensor_reduce(
                                stat[:, g:g + 1], cur[:],
                                axis=mybir.AxisListType.X, op=op)
                elif parts == "gpsimd_only":
                    for g in range(NGL):
                        nc.gpsimd.tensor_tensor(
                            gt1[:].rearrange("p (r w) -> p r w", w=W),
                            mxb[:, 0:HALF, :], mxb[:, HALF:TPGL, :],
                            mybir.AluOpType.max)
                else:  # pe_only
                    for g in range(NGL):
                        pg = psump.tile([IPG, W], f32, tag="pg")
                        for t in range(TPGL):
                            nc.tensor.matmul(pg[:], mrb[:, t, :],
                                             mxb[:, t, :],
                                             start=(t == 0),
                                             stop=(t == TPGL - 1))
                        nc.vector.tensor_copy(sbps[:, g * W:(g + 1) * W],
                                              pg[:])

            gt1 = None
            if parts == "gpsimd_only":
                gt1 = constp.tile([P, HALF * W], bf16, tag="gt1")
            if micro:
                mxg = constp.tile([P, TPGL, W], f32, tag="mxg")
                nc.sync.dma_start(
                    mxg[:], xview[:, 0, :].rearrange("p (r w) -> p r w", w=W))
                mxb = constp.tile([P, TPGL, W], bf16, tag="mxb")
                nc.scalar.activation(mxb[:], mxg[:],
                                     mybir.ActivationFunctionType.Copy)
                mrb = constp.tile([P, TPGL, IPG], bf16, tag="mrb")
                nc.sync.dma_start(mrb[:],
                                  rb[:, 0:TPGL * IPG]
                                  .rearrange("p (t j) -> p t j", j=IPG))

            loop_cm = (tc.For_i(0, loop_k, 1) if loop_k > 1
                       else contextlib.nullcontext())
            with loop_cm:
                if micro:
                    micro_body()
                else:
                    for g in range(NGL):
                        group_body(g)

            if micro:
                # consume whatever the micro loop produced
                if parts == "cast_only":
                    nc.sync.dma_start(sums_o[:], sumstat[:])
                elif parts == "trees_only":
                    nc.sync.dma_start(mins_o[:], minstat[:])
                    nc.sync.dma_start(maxs_o[:], maxstat[:])
                elif parts == "gpsimd_only":
                    nc.sync.dma_start(t1_o[:], gt1[:])
                else:
                    nc.sync.dma_start(ps_o[:], sbps[:])
            if not micro and do_mm:
                nc.sync.dma_start(mins_o[:], minstat[:])
                nc.sync.dma_start(maxs_o[:], maxstat[:])
            if not micro and do_cast:
                nc.sync.dma_start(sums_o[:], sumstat[:])
            if not micro and do_pe:
                nc.sync.dma_start(ps_o[:], sbps[:])

    nc.compile()
    return nc


def _host_prep_v3(x_np, bboxes, tpg=28):
    TPGL = tpg
    NGL = NT // TPGL
    IPG = BL // NGL
    bb = np.asarray(bboxes).astype(np.int64)
    x1 = np.clip(bb[:, 0], 0, W - 1)
    y1 = np.clip(bb[:, 1], 0, H - 1)
    x2 = np.clip(bb[:, 2], 0, W - 1)
    y2 = np.clip(bb[:, 3], 0, H - 1)
    yy = np.arange(H)
    xx = np.arange(W)
    rbox = (yy[None, :] >= y1[:, None]) & (yy[None, :] <= y2[:, None])  # (B,H)
    cbox = (xx[None, :] >= x1[:, None]) & (xx[None, :] <= x2[:, None])  # (B,W)

    # local flat row of (g,p,t) = g*P*TPG + p*TPG + t; every partition line
    # stays inside image  img_l = g*IPG + p//8,  rows 28*(p%8) + t.
    p_i = np.arange(P)[:, None, None]
    g_i = np.arange(NGL)[None, :, None]
    t_i = np.arange(TPGL)[None, None, :]
    img_l = g_i * IPG + p_i // (P // IPG)                 # (P, NG, TPG)
    within = (TPGL * (p_i % (P // IPG)) + t_i)
    onehot = (p_i[..., None] // (P // IPG) ==
              np.arange(IPG)[None, None, None, :])        # (P,1,1,IPG)

    in_maps = []
    for c in range(NCORES):
        Rm = rbox[img_l + c * BL, within]                 # (P, NG, TPG)
        rb_np = (Rm[..., None] & onehot).astype(ml_dtypes.bfloat16)
        in_maps.append({
            "x": np.ascontiguousarray(x_np[c * BL:(c + 1) * BL]),
            "rb": np.ascontiguousarray(
                rb_np.reshape(P, NGL * TPGL * IPG)),
        })
    return in_maps, rbox, cbox


def _host_combine_v3(results, rbox, cbox, tpg=28, subsplit=1):
    TPGL = tpg
    NGL = NT // TPGL
    IPG = BL // NGL
    PPI = P // IPG                                        # partitions/image
    SPL = max(subsplit, 1)
    mn = np.empty(B)
    mx = np.empty(B)
    S = np.empty(B)
    Sbox = np.empty(B)

    for c in range(NCORES):
        r = results[c]
        # stat[p, g] belongs to image g*IPG + p//PPI
        # -> [NG, IPG, PPI] with i = g*IPG + j
        mn[c * BL:(c + 1) * BL] = (
            r["mins"].T.reshape(NGL, IPG, PPI).min(2).reshape(BL))
        mx[c * BL:(c + 1) * BL] = (
            r["maxs"].T.reshape(NGL, IPG, PPI).max(2).reshape(BL))
        S[c * BL:(c + 1) * BL] = (
            r["sums"].astype(np.float64).reshape(P, NGL, SPL).sum(2).T
            .reshape(NGL, IPG, PPI).sum(2).reshape(BL))
        # ps[j, g*W + w] = box-row col sum for image g*IPG + j
        ps = (r["ps"].astype(np.float64).reshape(IPG, NGL, W)
              .transpose(1, 0, 2).reshape(BL, W))
        cb = cbox[c * BL:(c + 1) * BL]
        Sbox[c * BL:(c + 1) * BL] = (ps * cb).sum(1)

    nbox = rbox.sum(1).astype(np.float64) * cbox.sum(1).astype(np.float64)
    d = mx - mn + EPS
    sumA = (S - NPIX * mn) / d
    sumAbox = (Sbox - nbox * mn) / d
    loss = (sumA - sumAbox) / (sumA + EPS)
    return np.float32(loss.mean())


def _host_prep_v2(x_np, bboxes):
    IPG = BL // NG
    bb = np.asarray(bboxes).astype(np.int64)
    x1 = np.clip(bb[:, 0], 0, W - 1)
    y1 = np.clip(bb[:, 1], 0, H - 1)
    x2 = np.clip(bb[:, 2], 0, W - 1)
    y2 = np.clip(bb[:, 3], 0, H - 1)
    yy = np.arange(H)
    xx = np.arange(W)
    rbox = (yy[None, :] >= y1[:, None]) & (yy[None, :] <= y2[:, None])
    cbox = (xx[None, :] >= x1[:, None]) & (xx[None, :] <= x2[:, None])

    # local row of (g, p, t) is g*3584 + 28p + t; image-in-group = p // 8
    p_idx = np.arange(P)[:, None, None]
    g_idx = np.arange(NG)[None, :, None]
    t_idx = np.arange(TPG)[None, None, :]
    row = g_idx * (P * TPG) + p_idx * TPG + t_idx          # (P, NG, TPG)
    img_l = row // H                                       # local image
    within = row % H
    onehot = (p_idx[..., None] // 8 ==
              np.arange(IPG)[None, None, None, :])         # (P,1,1,IPG)

    in_maps = []
    for c in range(NCORES):
        Rm = rbox[img_l + c * BL, within]                  # (P, NG, TPG)
        full = np.zeros((P, NG, TPG, 2 * IPG), np.float32)
        full[..., :IPG] = onehot * Rm[..., None]
        full[..., IPG:] = np.broadcast_to(
            onehot, (P, NG, TPG, IPG)).astype(np.float32)
        rl_np = np.ascontiguousarray(
            full.reshape(P, NG * TPG * 2 * IPG)).astype(ml_dtypes.bfloat16)
        in_maps.append({
            "x": np.ascontiguousarray(x_np[c * BL:(c + 1) * BL]),
            "rl": rl_np,
        })
    return in_maps, rbox, cbox


def _host_combine_v2(results, rbox, cbox):
    IPG = BL // NG
    mn = np.empty(B)
    mx = np.empty(B)
    S = np.empty(B)
    Sbox = np.empty(B)

    for c in range(NCORES):
        r = results[c]
        # row g*3584 + 28p + t  ->  minstat[p, g*28 + t]
        rows_min = (r["mins"].reshape(P, NG, TPG).transpose(1, 0, 2)
                    .reshape(RPC))
        rows_max = (r["maxs"].reshape(P, NG, TPG).transpose(1, 0, 2)
                    .reshape(RPC))
        mn[c * BL:(c + 1) * BL] = rows_min.reshape(BL, H).min(1)
        mx[c * BL:(c + 1) * BL] = rows_max.reshape(BL, H).max(1)

        ps = r["ps"].reshape(HW2, NG, 2, 2 * IPG).astype(np.float64)
        pst = ps.transpose(1, 3, 2, 0)                     # (NG, 2*IPG, 2, HW2)
        box_part = pst[:, :IPG].reshape(BL, 2, HW2)        # l = g*IPG + j
        tot_part = pst[:, IPG:].reshape(BL, 2, HW2)
        cb = cbox[c * BL:(c + 1) * BL].reshape(BL, 2, HW2)
        Sbox[c * BL:(c + 1) * BL] = (box_part * cb).sum((1, 2))
        S[c * BL:(c + 1) * BL] = tot_part.sum((1, 2))

    nbox = rbox.sum(1).astype(np.float64) * cbox.sum(1).astype(np.float64)
    d = mx - mn + EPS
    sumA = (S - NPIX * mn) / d
    sumAbox = (Sbox - nbox * mn) / d
    loss = (sumA - sumAbox) / (sumA + EPS)
    return np.float32(loss.mean())


def _host_prep(x_np, bboxes):
    """Build per-core input maps."""
    bb = np.asarray(bboxes).astype(np.int64)
    x1 = np.clip(bb[:, 0], 0, W - 1)
    y1 = np.clip(bb[:, 1], 0, H - 1)
    x2 = np.clip(bb[:, 2], 0, W - 1)
    y2 = np.clip(bb[:, 3], 0, H - 1)
    yy = np.arange(H)
    xx = np.arange(W)
    rbox = (yy[None, :] >= y1[:, None]) & (yy[None, :] <= y2[:, None])  # (B,H)
    cbox = (xx[None, :] >= x1[:, None]) & (xx[None, :] <= x2[:, None])  # (B,W)

    img_of = np.repeat(np.arange(BL), H).reshape(NT, P)   # local img per row
    within = np.tile(np.arange(H), BL).reshape(NT, P)
    isA = img_of == img_of[:, 0:1]
    isB = ~isA

    in_maps = []
    for c in range(NCORES):
        gimg = img_of + c * BL
        Rm = rbox[gimg, within]                            # (NT, P) bool
        rl4 = np.stack([Rm & isA, Rm & isB, isA, isB], axis=2)  # (NT,P,4)
        rl_np = np.ascontiguousarray(
            rl4.transpose(1, 0, 2).reshape(P, NT * 4)).astype(ml_dtypes.bfloat16)
        in_maps.append({
            "x": np.ascontiguousarray(x_np[c * BL:(c + 1) * BL]),
            "rl": rl_np,
        })
    return in_maps, rbox, cbox


def _host_combine(results, rbox, cbox):
    mn = np.empty(B)
    mx = np.empty(B)
    S = np.zeros(B)
    Sbox = np.zeros(B)

    tiles = np.arange(NT)
    imgA_l = (P * tiles) // H                   # local first image of tile
    imgB_l = np.minimum(imgA_l + 1, BL - 1)     # clipped; contribution is 0
                                                # when the tile has no imgB rows
    cols = (np.arange(2)[:, None] * HW2 + np.arange(HW2)[None, :])  # (2,HW2)

    for c in range(NCORES):
        r = results[c]
        rows_min = r["mins"].T.reshape(RPC)     # row 128s+p -> mins[p,s]
        rows_max = r["maxs"].T.reshape(RPC)
        mn[c * BL:(c + 1) * BL] = rows_min.reshape(BL, H).min(1)
        mx[c * BL:(c + 1) * BL] = rows_max.reshape(BL, H).max(1)

        ps = r["ps"].reshape(HW2, NG, TPG, 2, 4).astype(np.float64)
        # -> (NT, 2, HW2, 4)
        pst = ps.transpose(1, 2, 3, 0, 4).reshape(NT, 2, HW2, 4)
        gA = imgA_l + c * BL
        gB = imgB_l + c * BL
        cA = cbox[gA][:, cols]                  # (NT, 2, HW2)
        cB = cbox[gB][:, cols]
        sboxA = (pst[..., 0] * cA).sum((1, 2))
        sboxB = (pst[..., 1] * cB).sum((1, 2))
        sA = pst[..., 2].sum((1, 2))
        sB = pst[..., 3].sum((1, 2))
        np.add.at(Sbox, gA, sboxA)
        np.add.at(Sbox, gB, sboxB)
        np.add.at(S, gA, sA)
        np.add.at(S, gB, sB)

    nbox = rbox.sum(1).astype(np.float64) * cbox.sum(1).astype(np.float64)
    d = mx - mn + EPS
    sumA = (S - NPIX * mn) / d
    sumAbox = (Sbox - nbox * mn) / d
    loss = (sumA - sumAbox) / (sumA + EPS)
    return np.float32(loss.mean())


BUILDER = "v3"
V3_CFG = dict(tpg=28, subsplit=4, tail_half_trees=True)


def kernel(attn_map, bboxes):
    global _compiled, LAST_RESULT
    from concourse.bass_utils import run_bass_kernel_spmd

    if _compiled is None:
        _compiled = (_build_module_v3(**V3_CFG) if BUILDER == "v3"
                     else _build_module())
    nc = _compiled

    x_np = np.ascontiguousarray(np.asarray(attn_map, dtype=np.float32))
    if BUILDER == "v3":
        in_maps, rbox, cbox = _host_prep_v3(x_np, bboxes, tpg=V3_CFG["tpg"])
    else:
        in_maps, rbox, cbox = _host_prep(x_np, bboxes)
    res = run_bass_kernel_spmd(nc, in_maps, list(range(NCORES)))
    LAST_RESULT = res
    if BUILDER == "v3":
        return _host_combine_v3(res.results, rbox, cbox,
                                tpg=V3_CFG["tpg"],
                                subsplit=V3_CFG["subsplit"])
    return _host_combine(res.results, rbox, cbox)

